# revision 1
# baseline (speedup 1.0000x reference)
"""Self-contained Trainium2 Bass kernel for single-head full-dim attention.

Reference computation (fp32 jax):
    q  = x @ Wq                      # [B, Nq, D]
    kv = y @ Wkv                     # [B, Nkv, 2D] -> k, v
    attn = softmax(q * D^-0.5 @ k^T) # [B, Nq, Nkv]
    out  = attn @ v                  # [B, Nq, D]
with B=4, Nq=Nkv=2048, D=1024.

Distribution: data parallel over 8 NeuronCores, shard = (batch b,
kv-half s).  Each core computes t for ALL 2048 queries of its batch
(cheap, duplicated across the pair), V for its 1024 keys, the
2048x1024 exp-score block, and the UNNORMALIZED output block
out'_s = exp(S_s) @ v_s plus the partial softmax denominator
Z_s = sum_k exp(S_s).  The host combines the two halves:
out = (out'_0 + out'_1) / (Z_0 + Z_1).  This avoids both collectives
and the (2x more expensive) duplicated K/V compute of a query-sharded
layout.

Algebraic fold: scores = (q*scale) @ k^T = x @ (scale*Wq@Wk^T) @ y^T.
The host precomputes W* = scale*Wq@Wk^T once (fp32, 2 GFLOP), so the
device never computes K at all: t = x @ W*, scoresT = y @ t^T with the
RAW y features as the contraction — the same yT tiles already loaded
for the V projection.  This removes 1/8 of all device matmuls.

Layout trick: everything on-chip is computed transposed
([feature, token]) so the TensorEngine can contract along partitions
without any on-chip transposes.  All matmul operands are bf16 (fp32
PSUM accumulation).  Softmax uses exp without max-subtraction (scores
~ N(0,1) by construction; fp32 exp is safe) on the scalar engine; Z is
a vector-engine add-tree plus 4 ones-matmuls issued AFTER the output
phase so the PE never waits on the tree.
"""

import numpy as np
import ml_dtypes

import concourse.bass as bass
import concourse.mybir as mybir
import concourse.tile as tile
from concourse.bass import ds
from concourse.bass_utils import run_bass_kernel_spmd

DIM = 1024
B = 4
NQ = 2048
NKV = 2048
N_CORES = 8
NKV_SHARD = 1024  # keys per core

BF16 = mybir.dt.bfloat16
F32 = mybir.dt.float32
NP_BF16 = ml_dtypes.bfloat16

N_WARM = 20


def _split_sync_waits(nc, max_waits: int = 1):
    """walrus in this toolchain rejects instructions carrying more than one
    sem wait ("Too many sync wait commands").  Hoist extra waits onto
    preceding same-engine NOPs: the engine dispatches in order, so waiting
    just before the instruction is semantically identical (at worst it
    delays issue slightly)."""
    import bass_rust as _bass_rust

    for f in nc.m.functions:
        for bb in f.blocks:
            insts = list(bb.instructions)
            out = []
            changed = False
            for inst in insts:
                si = getattr(inst, "sync_info", None)
                waits = list(si.on_wait) if si is not None and si.on_wait else []
                if len(waits) > max_waits:
                    changed = True
                    extra, keep = waits[:-max_waits], waits[-max_waits:]
                    for k in range(0, len(extra), max_waits):
                        nop = mybir.InstNoOp(
                            name=f"{inst.name}_sw{k}", engine=inst.engine,
                            ins=[], outs=[],
                        )
                        nop.sync_info = _bass_rust.SyncInfo(
                            on_wait=extra[k : k + max_waits], on_update=[]
                        )
                        out.append(nop)
                    si.on_wait = keep
                    inst.sync_info = si
                out.append(inst)
            if changed:
                bb.instructions = out


def build_attention_nc():
    """Build the per-core Bass graph (identical on all 8 cores)."""
    nc = bass.Bass()

    # DRAM parameters (per-core shards, host-prepped layouts; all bf16
    # except the f32 outputs).
    xT_d = nc.declare_dram_parameter("xT", [DIM, NQ], BF16, isOutput=False)
    yT_d = nc.declare_dram_parameter("yT", [DIM, NKV_SHARD], BF16, isOutput=False)
    # W* slabs pre-arranged so each DMA is per-partition contiguous:
    # ws[j, p, c, m] = W*[c*128+p, j*128+m]
    ws_d = nc.declare_dram_parameter("ws", [8, 128, 8, 128], BF16, isOutput=False)
    # Wv row chunks: wv[c] = Wv[c*128:(c+1)*128, :]
    wv_d = nc.declare_dram_parameter("wv", [8, 128, DIM], BF16, isOutput=False)
    out_d = nc.declare_dram_parameter("out", [NQ, DIM], F32, isOutput=True)
    z_d = nc.declare_dram_parameter("zout", [1, NQ], F32, isOutput=True)

    with tile.TileContext(nc) as tc:
        # Long-lived pool: on-chip intermediates live to the end.
        L = tc.alloc_tile_pool(name="L", bufs=1)
        pm = tc.alloc_tile_pool(name="pm", bufs=1, space="PSUM")
        # Transient input pools, released once consumed (LIFO: t2 first).
        t1 = tc.alloc_tile_pool(name="t1", bufs=1)  # xtc + ws slabs
        t2 = tc.alloc_tile_pool(name="t2", bufs=1)  # wv chunks

        # ---- HAM warm-up: dummy matmuls on a zeroed scratch tile run
        # during the otherwise-idle input-DMA window, flipping the PE clock
        # gate to 8/8 (2.4GHz) before the first real matmul arrives.
        wsc = t1.tile([128, 512], BF16, name="warm", tag="warm", bufs=1)
        nc.vector.memset(wsc[:], 0.0)
        wps = pm.tile([128, 512], F32, name="wps", tag="warm", bufs=1)
        for w in range(N_WARM):
            nc.tensor.matmul(
                wps[:], lhsT=wsc[:, 0:128], rhs=wsc[:],
                start=(w == 0), stop=(w == N_WARM - 1),
            )

        # ---- Input DMAs, most-urgent first.  P3 (V projection) runs
        # first and is dd-major, so its first 8 groups need only yT (2MB)
        # plus the LEFT column-half of Wv (1MB): 3MB before the PE can
        # stream (the 16 DMA queues drain in parallel at ~300GB/s
        # aggregate, so bytes-before-first-chain is what sets the head).
        ytr = yT_d.rearrange("(c p) n -> c p n", p=128)
        ytc, wvc = [], []
        for c in range(8):
            t = L.tile([128, NKV_SHARD], BF16, name=f"yt{c}", tag="yt", bufs=8)
            nc.sync.dma_start(out=t[:], in_=ytr[c])
            ytc.append(t)
            w = t2.tile([128, DIM], BF16, name=f"wv{c}", tag="wv", bufs=8)
            nc.sync.dma_start(out=w[:, 0:512], in_=wv_d[c][:, 0:512])
            wvc.append(w)
        for c in range(8):
            nc.sync.dma_start(out=wvc[c][:, 512:1024], in_=wv_d[c][:, 512:1024])
        ws_slabs = [
            t1.tile([128, 8, 128], BF16, name=f"ws{j}", tag="ws", bufs=8)
            for j in range(8)
        ]
        nc.sync.dma_start(out=ws_slabs[0][:], in_=ws_d[0])
        xtr = xT_d.rearrange("(c p) n -> c p n", p=128)
        xtc = []
        for c in range(8):
            t = t1.tile([128, NQ], BF16, name=f"xt{c}", tag="xt", bufs=8)
            nc.sync.dma_start(out=t[:], in_=xtr[c])
            xtc.append(t)
        for j in range(1, 8):
            nc.sync.dma_start(out=ws_slabs[j][:], in_=ws_d[j])

        # ---- P3: v[nkv, do] = sum_d yT[d, nkv] * Wv[d, do] --------------
        # dd-major: the dd=0 groups only touch the left Wv half.
        vt = [L.tile([128, DIM], BF16, name=f"v{i}", tag="v", bufs=8) for i in range(8)]
        for dd in range(2):  # d_out 512-chunk
            for i in range(8):  # nkv 128-tile
                ps = pm.tile([128, 512], F32, name=f"psv{i}_{dd}", tag="mm", bufs=4)
                for c in range(8):
                    nc.tensor.matmul(
                        ps[:],
                        lhsT=ytc[c][:, ds(i * 128, 128)],
                        rhs=wvc[c][:, ds(dd * 512, 512)],
                        start=(c == 0),
                        stop=(c == 7),
                    )
                nc.any.tensor_copy(vt[i][:, ds(dd * 512, 512)], ps[:])
        t2.release()

        # ---- P1: tT[e, nq] = sum_d W*[d, e] * xT[d, nq] ------------------
        qt = [L.tile([128, NQ], BF16, name=f"qt{j}", tag="qt", bufs=8) for j in range(8)]
        for j in range(8):  # e 128-chunk
            for q in range(4):  # nq 512-chunk
                ps = pm.tile([128, 512], F32, name=f"psq{j}_{q}", tag="mm", bufs=4)
                for c in range(8):  # d chunk (contraction)
                    nc.tensor.matmul(
                        ps[:],
                        lhsT=ws_slabs[j][:, c, :],
                        rhs=xtc[c][:, ds(q * 512, 512)],
                        start=(c == 0),
                        stop=(c == 7),
                    )
                nc.any.tensor_copy(qt[j][:, ds(q * 512, 512)], ps[:])
        t1.release()

        # ---- P4: expT[nkv, nq] = exp(sum_e yT[e,nkv] * tT[e,nq]) --------
        # (contraction over raw y-features e: y appears directly, no K!)
        et = [L.tile([128, NQ], BF16, name=f"e{i}", tag="et", bufs=8) for i in range(8)]
        for i in range(8):  # nkv 128-tile
            for q in range(4):  # nq 512-chunk
                ps = pm.tile([128, 512], F32, name=f"pse{i}_{q}", tag="mm", bufs=4)
                for c in range(8):  # e chunk (contraction)
                    nc.tensor.matmul(
                        ps[:],
                        lhsT=ytc[c][:, ds(i * 128, 128)],
                        rhs=qt[c][:, ds(q * 512, 512)],
                        start=(c == 0),
                        stop=(c == 7),
                    )
                nc.scalar.activation(
                    et[i][:, ds(q * 512, 512)],
                    ps[:],
                    mybir.ActivationFunctionType.Exp,
                )

        # ---- Z add-tree on the (otherwise idle) vector engine: collapse
        # the 8 et tiles to one f32 [128, NQ]; runs concurrently with the
        # P4/P7 matmul stream (gated only on et readiness).
        t3 = tc.alloc_tile_pool(name="t3", bufs=1)
        s0 = [t3.tile([128, NQ], F32, name=f"es0_{h}", tag="es", bufs=3) for h in range(2)]
        nc.vector.tensor_add(s0[0][:], et[0][:], et[1][:])
        nc.vector.tensor_add(s0[1][:], et[2][:], et[3][:])
        s1 = t3.tile([128, NQ], F32, name="es1", tag="es2", bufs=2)
        nc.vector.tensor_add(s1[:], s0[0][:], s0[1][:])
        s0b = [t3.tile([128, NQ], F32, name=f"es0b_{h}", tag="es", bufs=3) for h in range(2)]
        nc.vector.tensor_add(s0b[0][:], et[4][:], et[5][:])
        nc.vector.tensor_add(s0b[1][:], et[6][:], et[7][:])
        s2 = t3.tile([128, NQ], F32, name="es2", tag="es2", bufs=2)
        nc.vector.tensor_add(s2[:], s0b[0][:], s0b[1][:])
        stot = t3.tile([128, NQ], F32, name="estot", tag="es", bufs=3)
        nc.vector.tensor_add(stot[:], s1[:], s2[:])
        ones = L.tile([128, 1], F32, name="ones", bufs=1)
        nc.vector.memset(ones[:], 1.0)

        # ---- P7: out'[nq, do] = sum_nkv expT[nkv,nq] * v[nkv,do] --------
        # The Z ones-matmuls ([1,512] partition reductions of the vector
        # add-tree result) slot in mid-stream at t==8: stot is ready ~4us
        # after P4 ends, long before then, so the PE never stalls and the
        # tail only carries the last out-tile's copy+DMA.
        for t in range(16):  # nq 128-tile
            for dd in range(2):  # d_out 512-chunk
                if t == 15 and dd == 1:
                    # Tail: two N=256 groups so copy+DMA of the first half
                    # overlaps the second half's matmuls, halving the drain.
                    for h in range(2):
                        psh = pm.tile([128, 256], F32, name=f"psoL{h}", tag="mmL", bufs=2)
                        for i in range(8):
                            nc.tensor.matmul(
                                psh[:],
                                lhsT=et[i][:, ds(t * 128, 128)],
                                rhs=vt[i][:, ds(dd * 512 + h * 256, 256)],
                                start=(i == 0),
                                stop=(i == 7),
                            )
                        obh = L.tile([128, 256], F32, name=f"oL{h}", tag="oL", bufs=2)
                        nc.any.tensor_copy(obh[:], psh[:])
                        nc.sync.dma_start(
                            out=out_d[ds(t * 128, 128), ds(dd * 512 + h * 256, 256)],
                            in_=obh[:],
                        )
                    continue
                ps = pm.tile([128, 512], F32, name=f"pso{t}_{dd}", tag="mm", bufs=4)
                for i in range(8):  # nkv contraction
                    nc.tensor.matmul(
                        ps[:],
                        lhsT=et[i][:, ds(t * 128, 128)],
                        rhs=vt[i][:, ds(dd * 512, 512)],
                        start=(i == 0),
                        stop=(i == 7),
                    )
                ob = L.tile([128, 512], F32, name=f"o{t}_{dd}", tag="o", bufs=3)
                nc.any.tensor_copy(ob[:], ps[:])
                nc.sync.dma_start(
                    out=out_d[ds(t * 128, 128), ds(dd * 512, 512)], in_=ob[:]
                )
            if t == 8:
                for q in range(4):
                    psz = pm.tile([1, 512], F32, name=f"psz{q}", tag="zr", bufs=1)
                    nc.tensor.matmul(
                        psz[:],
                        lhsT=ones[:],
                        rhs=stot[:, ds(q * 512, 512)],
                        start=True,
                        stop=True,
                    )
                    zrow = L.tile([1, 512], F32, name=f"zrow{q}", tag="zrow", bufs=2)
                    nc.any.tensor_copy(zrow[:], psz[:])
                    nc.sync.dma_start(out=z_d[0:1, ds(q * 512, 512)], in_=zrow[:])
        t3.release()
        pm.release()
        L.release()

    _split_sync_waits(nc)
    return nc


_NC_CACHE = {}


def _get_nc():
    if "nc" not in _NC_CACHE:
        _NC_CACHE["nc"] = build_attention_nc()
    return _NC_CACHE["nc"]


def make_in_maps(x, y, Wq, Wkv):
    """Host-side sharding + layout prep. Returns in_maps for cores 0-7."""
    scale = DIM ** (-0.5)
    wkv = np.asarray(Wkv, np.float32)
    # W* = scale * Wq @ Wk^T (fp32, once) -> bf16 slabs
    wstar = (np.asarray(Wq, np.float32) * scale) @ wkv[:, :DIM].T
    # ws[j, p, c, m] = W*[c*128+p, j*128+m]
    ws = np.ascontiguousarray(
        wstar.astype(NP_BF16).reshape(8, 128, 8, 128).transpose(2, 1, 0, 3)
    )
    wv = np.ascontiguousarray(wkv[:, DIM:].astype(NP_BF16).reshape(8, 128, DIM))

    x = np.asarray(x, np.float32)
    y = np.asarray(y, np.float32)
    in_maps = []
    for core in range(N_CORES):
        b, s = divmod(core, 2)
        xT = np.ascontiguousarray(x[b].T).astype(NP_BF16)
        yT = np.ascontiguousarray(
            y[b, s * NKV_SHARD : (s + 1) * NKV_SHARD, :].T
        ).astype(NP_BF16)
        in_maps.append({"xT": xT, "yT": yT, "ws": ws, "wv": wv})
    return in_maps


def run_sharded(x, y, Wq, Wkv, trace=False, tmpdir=None):
    """Run the SPMD kernel; returns (full_output, BassKernelResults)."""
    nc = _get_nc()
    in_maps = make_in_maps(x, y, Wq, Wkv)
    try:
        res = run_bass_kernel_spmd(
            nc, in_maps, core_ids=list(range(N_CORES)), trace=trace, tmpdir=tmpdir
        )
    except Exception:
        # one retry: transient NRT device states (e.g. a previous crashed
        # load) usually clear on the next attempt
        res = run_bass_kernel_spmd(
            nc, in_maps, core_ids=list(range(N_CORES)), trace=trace, tmpdir=tmpdir
        )
    out = np.empty((B, NQ, DIM), np.float32)
    for b in range(B):
        r0, r1 = res.results[2 * b], res.results[2 * b + 1]
        num = r0["out"] + r1["out"]
        z = (r0["zout"] + r1["zout"])[0]
        out[b] = num / z[:, None]
    return out, res


def kernel(x, y, Wq, Wkv):
    out, _ = run_sharded(x, y, Wq, Wkv)
    return out



# revision 6
# speedup vs baseline: 1.2776x; 1.2776x over previous
"""Self-contained Trainium2 Bass kernel for single-head full-dim attention.

Reference computation (fp32 jax):
    q  = x @ Wq                      # [B, Nq, D]
    kv = y @ Wkv                     # [B, Nkv, 2D] -> k, v
    attn = softmax(q * D^-0.5 @ k^T) # [B, Nq, Nkv]
    out  = attn @ v                  # [B, Nq, D]
with B=4, Nq=Nkv=2048, D=1024.

Distribution: data parallel over 8 NeuronCores, shard = (batch b,
kv-half s).  Each core computes t for ALL 2048 queries of its batch
(cheap, duplicated across the pair), V for its 1024 keys, the
2048x1024 exp-score block, and the UNNORMALIZED output block
out'_s = exp(S_s) @ v_s plus the partial softmax denominator
Z_s = sum_k exp(S_s).  The host combines the two halves:
out = (out'_0 + out'_1) / (Z_0 + Z_1).  This avoids both collectives
and the (2x more expensive) duplicated K/V compute of a query-sharded
layout.

Algebraic fold: scores = (q*scale) @ k^T = x @ (scale*Wq@Wk^T) @ y^T.
The host precomputes W* = scale*Wq@Wk^T once (fp32, 2 GFLOP), so the
device never computes K at all: t = x @ W*, scoresT = y @ t^T with the
RAW y features as the contraction — the same yT tiles already loaded
for the V projection.  This removes 1/8 of all device matmuls.

Layout trick: everything on-chip is computed transposed
([feature, token]) so the TensorEngine can contract along partitions
without any on-chip transposes.  All matmul operands are bf16 (fp32
PSUM accumulation).  Softmax uses exp without max-subtraction (scores
~ N(0,1) by construction; fp32 exp is safe) on the scalar engine; Z is
a vector-engine add-tree plus 4 ones-matmuls issued AFTER the output
phase so the PE never waits on the tree.
"""

import numpy as np
import ml_dtypes

import concourse.bass as bass
import concourse.mybir as mybir
import concourse.tile as tile
from concourse.bass import ds
from concourse.bass_utils import run_bass_kernel_spmd

DIM = 1024
B = 4
NQ = 2048
NKV = 2048
N_CORES = 8
NKV_SHARD = 1024  # keys per core

BF16 = mybir.dt.bfloat16
F32 = mybir.dt.float32
NP_BF16 = ml_dtypes.bfloat16

N_WARM = 20


def _split_sync_waits(nc, max_waits: int = 1):
    """walrus in this toolchain rejects instructions carrying more than one
    sem wait ("Too many sync wait commands").  Hoist extra waits onto
    preceding same-engine NOPs: the engine dispatches in order, so waiting
    just before the instruction is semantically identical (at worst it
    delays issue slightly)."""
    import bass_rust as _bass_rust

    for f in nc.m.functions:
        for bb in f.blocks:
            insts = list(bb.instructions)
            out = []
            changed = False
            for inst in insts:
                si = getattr(inst, "sync_info", None)
                waits = list(si.on_wait) if si is not None and si.on_wait else []
                if len(waits) > max_waits:
                    changed = True
                    extra, keep = waits[:-max_waits], waits[-max_waits:]
                    for k in range(0, len(extra), max_waits):
                        nop = mybir.InstNoOp(
                            name=f"{inst.name}_sw{k}", engine=inst.engine,
                            ins=[], outs=[],
                        )
                        nop.sync_info = _bass_rust.SyncInfo(
                            on_wait=extra[k : k + max_waits], on_update=[]
                        )
                        out.append(nop)
                    si.on_wait = keep
                    inst.sync_info = si
                out.append(inst)
            if changed:
                bb.instructions = out


def build_attention_nc():
    """Build the per-core Bass graph (identical on all 8 cores)."""
    nc = bass.Bass()

    # DRAM parameters (per-core shards, host-prepped layouts; all bf16
    # except the f32 outputs).
    xT_d = nc.declare_dram_parameter("xT", [DIM, NQ], BF16, isOutput=False)
    yT_d = nc.declare_dram_parameter("yT", [DIM, NKV_SHARD], BF16, isOutput=False)
    # W* slabs pre-arranged so each DMA is per-partition contiguous:
    # ws[j, p, c, m] = W*[c*128+p, j*128+m]
    ws_d = nc.declare_dram_parameter("ws", [8, 128, 8, 128], BF16, isOutput=False)
    # Wv row chunks: wv[c] = Wv[c*128:(c+1)*128, :]
    wv_d = nc.declare_dram_parameter("wv", [8, 128, DIM], BF16, isOutput=False)
    # Outputs in bf16: the host pair-combine (sum + divide) happens in f32,
    # so bf16 only adds ~0.3% quantization on out' — well inside budget —
    # and halves the writeback DMA bytes (8MB -> 4MB per core).
    out_d = nc.declare_dram_parameter("out", [NQ, DIM], BF16, isOutput=True)
    z_d = nc.declare_dram_parameter("zout", [1, NQ], F32, isOutput=True)

    with tile.TileContext(nc) as tc:
        # Long-lived pool: on-chip intermediates live to the end.
        L = tc.alloc_tile_pool(name="L", bufs=1)
        pm = tc.alloc_tile_pool(name="pm", bufs=1, space="PSUM")
        # Transient input pools, released once consumed (LIFO: t2 first).
        t1 = tc.alloc_tile_pool(name="t1", bufs=1)  # xtc + ws slabs
        t2 = tc.alloc_tile_pool(name="t2", bufs=1)  # wv chunks

        # ---- HAM warm-up: dummy matmuls on a zeroed scratch tile run
        # during the otherwise-idle input-DMA window, flipping the PE clock
        # gate to 8/8 (2.4GHz) before the first real matmul arrives.
        wsc = t1.tile([128, 512], BF16, name="warm", tag="warm", bufs=1)
        nc.vector.memset(wsc[:], 0.0)
        wps = pm.tile([128, 512], F32, name="wps", tag="warm", bufs=1)
        for w in range(N_WARM):
            nc.tensor.matmul(
                wps[:], lhsT=wsc[:, 0:128], rhs=wsc[:],
                start=(w == 0), stop=(w == N_WARM - 1),
            )

        # ---- Input DMAs, most-urgent first.  P3 (V projection) runs
        # first and is dd-major, so its first 8 groups need only yT (2MB)
        # plus the LEFT column-half of Wv (1MB): 3MB before the PE can
        # stream (the 16 DMA queues drain in parallel at ~300GB/s
        # aggregate, so bytes-before-first-chain is what sets the head).
        ytr = yT_d.rearrange("(c p) n -> c p n", p=128)
        ytc, wvc = [], []
        for c in range(8):
            t = L.tile([128, NKV_SHARD], BF16, name=f"yt{c}", tag="yt", bufs=8)
            nc.sync.dma_start(out=t[:], in_=ytr[c])
            ytc.append(t)
            w = t2.tile([128, DIM], BF16, name=f"wv{c}", tag="wv", bufs=8)
            nc.sync.dma_start(out=w[:, 0:512], in_=wv_d[c][:, 0:512])
            wvc.append(w)
        for c in range(8):
            nc.sync.dma_start(out=wvc[c][:, 512:1024], in_=wv_d[c][:, 512:1024])
        ws_slabs = [
            t1.tile([128, 8, 128], BF16, name=f"ws{j}", tag="ws", bufs=8)
            for j in range(8)
        ]
        nc.sync.dma_start(out=ws_slabs[0][:], in_=ws_d[0])
        xtr = xT_d.rearrange("(c p) n -> c p n", p=128)
        xtc = []
        for c in range(8):
            t = t1.tile([128, NQ], BF16, name=f"xt{c}", tag="xt", bufs=8)
            nc.sync.dma_start(out=t[:], in_=xtr[c])
            xtc.append(t)
        for j in range(1, 8):
            nc.sync.dma_start(out=ws_slabs[j][:], in_=ws_d[j])

        # ---- P3: v[nkv, do] = sum_d yT[d, nkv] * Wv[d, do] --------------
        # dd-major: the dd=0 groups only touch the left Wv half.
        vt = [L.tile([128, DIM], BF16, name=f"v{i}", tag="v", bufs=8) for i in range(8)]
        for dd in range(2):  # d_out 512-chunk
            for i in range(8):  # nkv 128-tile
                ps = pm.tile([128, 512], F32, name=f"psv{i}_{dd}", tag="mm", bufs=4)
                for c in range(8):
                    nc.tensor.matmul(
                        ps[:],
                        lhsT=ytc[c][:, ds(i * 128, 128)],
                        rhs=wvc[c][:, ds(dd * 512, 512)],
                        start=(c == 0),
                        stop=(c == 7),
                    )
                nc.any.tensor_copy(vt[i][:, ds(dd * 512, 512)], ps[:])
        t2.release()

        # ---- P1: tT[e, nq] = sum_d W*[d, e] * xT[d, nq] ------------------
        qt = [L.tile([128, NQ], BF16, name=f"qt{j}", tag="qt", bufs=8) for j in range(8)]
        for j in range(8):  # e 128-chunk
            for q in range(4):  # nq 512-chunk
                ps = pm.tile([128, 512], F32, name=f"psq{j}_{q}", tag="mm", bufs=4)
                for c in range(8):  # d chunk (contraction)
                    nc.tensor.matmul(
                        ps[:],
                        lhsT=ws_slabs[j][:, c, :],
                        rhs=xtc[c][:, ds(q * 512, 512)],
                        start=(c == 0),
                        stop=(c == 7),
                    )
                nc.any.tensor_copy(qt[j][:, ds(q * 512, 512)], ps[:])
        t1.release()

        # ---- P4: expT[nkv, nq] = exp(sum_e yT[e,nkv] * tT[e,nq]) --------
        # (contraction over raw y-features e: y appears directly, no K!)
        et = [L.tile([128, NQ], BF16, name=f"e{i}", tag="et", bufs=8) for i in range(8)]
        for i in range(8):  # nkv 128-tile
            for q in range(4):  # nq 512-chunk
                ps = pm.tile([128, 512], F32, name=f"pse{i}_{q}", tag="mm", bufs=4)
                for c in range(8):  # e chunk (contraction)
                    nc.tensor.matmul(
                        ps[:],
                        lhsT=ytc[c][:, ds(i * 128, 128)],
                        rhs=qt[c][:, ds(q * 512, 512)],
                        start=(c == 0),
                        stop=(c == 7),
                    )
                nc.scalar.activation(
                    et[i][:, ds(q * 512, 512)],
                    ps[:],
                    mybir.ActivationFunctionType.Exp,
                )

        # ---- Z add-tree on the (otherwise idle) vector engine: collapse
        # the 8 et tiles to one bf16 [128, NQ]; runs concurrently with the
        # P4/P7 matmul stream (gated only on et readiness).  bf16 (not f32)
        # so the Z ones-matmuls below are plain single-pass bf16 MMs: f32
        # matmuls run LOW_HIGH 2-pass on the PE and each Z group was
        # measured to cost ~1.3us of stream disruption.  Z error from bf16
        # sums averages down ~sqrt(128) in the partition reduction.
        t3 = tc.alloc_tile_pool(name="t3", bufs=1)
        s0 = [t3.tile([128, NQ], BF16, name=f"es0_{h}", tag="es", bufs=3) for h in range(2)]
        nc.vector.tensor_add(s0[0][:], et[0][:], et[1][:])
        nc.vector.tensor_add(s0[1][:], et[2][:], et[3][:])
        s1 = t3.tile([128, NQ], BF16, name="es1", tag="es2", bufs=2)
        nc.vector.tensor_add(s1[:], s0[0][:], s0[1][:])
        s0b = [t3.tile([128, NQ], BF16, name=f"es0b_{h}", tag="es", bufs=3) for h in range(2)]
        nc.vector.tensor_add(s0b[0][:], et[4][:], et[5][:])
        nc.vector.tensor_add(s0b[1][:], et[6][:], et[7][:])
        s2 = t3.tile([128, NQ], BF16, name="es2", tag="es2", bufs=2)
        nc.vector.tensor_add(s2[:], s0b[0][:], s0b[1][:])
        stot = t3.tile([128, NQ], BF16, name="estot", tag="es", bufs=3)
        nc.vector.tensor_add(stot[:], s1[:], s2[:])
        ones = L.tile([128, 1], BF16, name="ones", bufs=1)
        nc.vector.memset(ones[:], 1.0)

        # ---- P7: out'[nq, do] = sum_nkv expT[nkv,nq] * v[nkv,do] --------
        # The Z ones-matmuls ([1,512] partition reductions of the vector
        # add-tree result) slot in mid-stream at t==8: stot is ready ~4us
        # after P4 ends, long before then, so the PE never stalls and the
        # tail only carries the last out-tile's copy+DMA.
        for t in range(16):  # nq 128-tile
            for dd in range(2):  # d_out 512-chunk
                if t == 15 and dd == 1:
                    # Tail: two N=256 groups so copy+DMA of the first half
                    # overlaps the second half's matmuls, halving the drain.
                    for h in range(2):
                        psh = pm.tile([128, 256], F32, name=f"psoL{h}", tag="mmL", bufs=2)
                        for i in range(8):
                            nc.tensor.matmul(
                                psh[:],
                                lhsT=et[i][:, ds(t * 128, 128)],
                                rhs=vt[i][:, ds(dd * 512 + h * 256, 256)],
                                start=(i == 0),
                                stop=(i == 7),
                            )
                        obh = L.tile([128, 256], BF16, name=f"oL{h}", tag="oL", bufs=2)
                        nc.any.tensor_copy(obh[:], psh[:])
                        nc.sync.dma_start(
                            out=out_d[ds(t * 128, 128), ds(dd * 512 + h * 256, 256)],
                            in_=obh[:],
                        )
                    continue
                ps = pm.tile([128, 512], F32, name=f"pso{t}_{dd}", tag="mm", bufs=4)
                for i in range(8):  # nkv contraction
                    nc.tensor.matmul(
                        ps[:],
                        lhsT=et[i][:, ds(t * 128, 128)],
                        rhs=vt[i][:, ds(dd * 512, 512)],
                        start=(i == 0),
                        stop=(i == 7),
                    )
                ob = L.tile([128, 512], BF16, name=f"o{t}_{dd}", tag="o", bufs=3)
                nc.any.tensor_copy(ob[:], ps[:])
                nc.sync.dma_start(
                    out=out_d[ds(t * 128, 128), ds(dd * 512, 512)], in_=ob[:]
                )
            if t == 8:
                for q in range(4):
                    psz = pm.tile([1, 512], F32, name=f"psz{q}", tag="zr", bufs=1)
                    nc.tensor.matmul(
                        psz[:],
                        lhsT=ones[:],
                        rhs=stot[:, ds(q * 512, 512)],
                        start=True,
                        stop=True,
                    )
                    zrow = L.tile([1, 512], F32, name=f"zrow{q}", tag="zrow", bufs=2)
                    nc.any.tensor_copy(zrow[:], psz[:])
                    nc.sync.dma_start(out=z_d[0:1, ds(q * 512, 512)], in_=zrow[:])
        t3.release()
        pm.release()
        L.release()

    _split_sync_waits(nc)
    return nc


_NC_CACHE = {}


def _get_nc():
    if "nc" not in _NC_CACHE:
        _NC_CACHE["nc"] = build_attention_nc()
    return _NC_CACHE["nc"]


def make_in_maps(x, y, Wq, Wkv):
    """Host-side sharding + layout prep. Returns in_maps for cores 0-7."""
    scale = DIM ** (-0.5)
    wkv = np.asarray(Wkv, np.float32)
    # W* = scale * Wq @ Wk^T (fp32, once) -> bf16 slabs
    wstar = (np.asarray(Wq, np.float32) * scale) @ wkv[:, :DIM].T
    # ws[j, p, c, m] = W*[c*128+p, j*128+m]
    ws = np.ascontiguousarray(
        wstar.astype(NP_BF16).reshape(8, 128, 8, 128).transpose(2, 1, 0, 3)
    )
    wv = np.ascontiguousarray(wkv[:, DIM:].astype(NP_BF16).reshape(8, 128, DIM))

    x = np.asarray(x, np.float32)
    y = np.asarray(y, np.float32)
    in_maps = []
    for core in range(N_CORES):
        b, s = divmod(core, 2)
        xT = np.ascontiguousarray(x[b].T).astype(NP_BF16)
        yT = np.ascontiguousarray(
            y[b, s * NKV_SHARD : (s + 1) * NKV_SHARD, :].T
        ).astype(NP_BF16)
        in_maps.append({"xT": xT, "yT": yT, "ws": ws, "wv": wv})
    return in_maps


def run_sharded(x, y, Wq, Wkv, trace=False, tmpdir=None):
    """Run the SPMD kernel; returns (full_output, BassKernelResults)."""
    nc = _get_nc()
    in_maps = make_in_maps(x, y, Wq, Wkv)
    try:
        res = run_bass_kernel_spmd(
            nc, in_maps, core_ids=list(range(N_CORES)), trace=trace, tmpdir=tmpdir
        )
    except Exception:
        # one retry: transient NRT device states (e.g. a previous crashed
        # load) usually clear on the next attempt
        res = run_bass_kernel_spmd(
            nc, in_maps, core_ids=list(range(N_CORES)), trace=trace, tmpdir=tmpdir
        )
    out = np.empty((B, NQ, DIM), np.float32)
    for b in range(B):
        r0, r1 = res.results[2 * b], res.results[2 * b + 1]
        num = r0["out"].astype(np.float32) + r1["out"].astype(np.float32)
        z = (r0["zout"].astype(np.float32) + r1["zout"].astype(np.float32))[0]
        out[b] = num / z[:, None]
    return out, res


def kernel(x, y, Wq, Wkv):
    out, _ = run_sharded(x, y, Wq, Wkv)
    return out

